# revision 1
# baseline (speedup 1.0000x reference)
"""GQA attention (B=2,S=1024,HID=2048,NH=32,NKV=8,HD=64) on 8 TRN2 cores.

Sharding: core c -> batch b=c//4, head-group g=c%4 (8 q heads / 2 kv heads).
Device computes, per core, partial out[b] = attn(heads of g) @ Wo[rows of g].
Host pre-transposes hidden_states, pre-casts to bf16, folds the 1/sqrt(d)
softmax scale into Wq, gathers RoPE tables by position_ids, and sums the 4
row-parallel Wo partials per batch at the end.

Device dataflow (all matmuls bf16 -> fp32 PSUM):
  qT = Wq_g.T @ hs.T         [512, 1024]   (4 chunks of 128)
  kT = Wk_rep.T @ hs.T       [2x128, 1024] (each kv head replicated to both
                                            64-row halves so scores matmuls
                                            are partition-aligned per q head)
  V  = hs @ Wv_g             [1024, 128]
  RoPE: rot_half as a 128x128 +-1 permutation matmul, then DVE combine
  per q-head h: S^T[kc] = kT_h[:,kc].T-matmul -> exp (ScalarE) -> bf16
                attn  += V_h[kc].T @ expS^T[kc]   (into rows h%2*64..)
                sums  += ones.T @ expS^T[kc]      (partition 0)
  normalize: recip = 1/sums; gpsimd partition_broadcast; DVE multiply
  out_partial = attnT.T @ Wo_g  -> DMA out [1024, 2048] f32
"""

import numpy as np
import ml_dtypes

import concourse.bass as bass
import concourse.bacc as bacc
import concourse.mybir as mybir
from concourse.tile import TileContext
from concourse.bass_utils import run_bass_kernel_spmd
from concourse.masks import make_identity

B, S, HID = 2, 1024, 2048
NH, NKV, HD = 32, 8, 64
G = 4                      # head groups (tensor-parallel degree per batch)
QH = NH // G               # 8 q heads per core
KVH = NKV // G             # 2 kv heads per core
QD = QH * HD               # 512
ROPE_BASE = 10000.0
BF16 = mybir.dt.bfloat16
F32 = mybir.dt.float32
NEG_BIG = float(np.finfo(np.float32).min)

LAST_RESULT = None
_CACHE = {}


def _build(use_mask: bool) -> bass.Bass:
    nc = bacc.Bacc(None, target_bir_lowering=False)
    hsT_d = nc.dram_tensor("hsT", [HID, S], BF16, kind="ExternalInput")
    wq_d = nc.dram_tensor("wq", [HID, QD], BF16, kind="ExternalInput")
    wk_d = nc.dram_tensor("wk", [HID, KVH * 2 * HD], BF16, kind="ExternalInput")
    wv_d = nc.dram_tensor("wv", [HID, KVH * HD], BF16, kind="ExternalInput")
    wo_d = nc.dram_tensor("wo", [QD, HID], BF16, kind="ExternalInput")
    cos_d = nc.dram_tensor("cos2", [128, S], F32, kind="ExternalInput")
    sin_d = nc.dram_tensor("sin2", [128, S], F32, kind="ExternalInput")
    perm_d = nc.dram_tensor("permT", [128, 128], BF16, kind="ExternalInput")
    if use_mask:
        mask_d = nc.dram_tensor("maskT", [S, S], BF16, kind="ExternalInput")
    out_d = nc.dram_tensor("out", [HID, S], F32, kind="ExternalOutput")

    KC = S // 128            # 8 k-token chunks
    TC = S // 128            # 8 token chunks
    HC = HID // 128          # 16 hidden chunks

    with TileContext(nc) as tc:
        with (
            tc.tile_pool(name="resid", bufs=1) as rp,
            tc.tile_pool(name="work", bufs=2) as wp,
            tc.tile_pool(name="exps", bufs=3) as ep,
            tc.tile_pool(name="outs", bufs=3) as op_,
        ):
            # ---- resident input tiles; hsT/wq interleaved per k-chunk so the
            # first projection matmul can start ~1.5us in (sim DMA is serial) ----
            permT = rp.tile([128, 128], BF16, tag="permT")
            nc.sync.dma_start(out=permT[:], in_=perm_d[:, :])
            cos2 = rp.tile([128, S], F32, tag="cos2")
            nc.sync.dma_start(out=cos2[:], in_=cos_d[:, :])
            sin2 = rp.tile([128, S], F32, tag="sin2")
            nc.sync.dma_start(out=sin2[:], in_=sin_d[:, :])
            hsT = []
            wq = []
            for k in range(HC):
                t = rp.tile([128, S], BF16, tag=f"hsT{k}")
                nc.sync.dma_start(out=t[:], in_=hsT_d[k * 128:(k + 1) * 128, :])
                hsT.append(t)
                t2 = rp.tile([128, QD], BF16, tag=f"wq{k}")
                nc.sync.dma_start(out=t2[:], in_=wq_d[k * 128:(k + 1) * 128, :])
                wq.append(t2)
            wk = rp.tile([128, HC * KVH * 2 * HD], BF16, tag="wk")
            nc.sync.dma_start(
                out=wk[:].rearrange("p (k m) -> p k m", k=HC),
                in_=wk_d[:, :].rearrange("(k p) m -> p k m", p=128),
            )
            wv = rp.tile([128, HC * KVH * HD], BF16, tag="wv")
            nc.sync.dma_start(
                out=wv[:].rearrange("p (k m) -> p k m", k=HC),
                in_=wv_d[:, :].rearrange("(k p) m -> p k m", p=128),
            )
            wo = rp.tile([128, (QD // 128) * HID], BF16, tag="wo")
            nc.sync.dma_start(
                out=wo[:].rearrange("p (k m) -> p k m", k=QD // 128),
                in_=wo_d[:, :].rearrange("(k p) m -> p k m", p=128),
            )
            if use_mask:
                maskT = rp.tile([128, KC * S], BF16, tag="maskT")
                nc.sync.dma_start(
                    out=maskT[:].rearrange("p (k q) -> p k q", k=KC),
                    in_=mask_d[:, :].rearrange("(k p) q -> p k q", p=128),
                )
            ones_col = rp.tile([128, 1], BF16, tag="ones")
            nc.any.memset(ones_col[:], 1.0)
            ident = rp.tile([128, 128], BF16, tag="ident")
            make_identity(nc, ident[:])

            # ---- persistent intermediates ----
            qrot = rp.tile([128, (QD // 128) * S], BF16, tag="qrot")
            krep = rp.tile([128, KVH * S], BF16, tag="krep")  # kv head i at cols i*S
            vaug = rp.tile([128, KC * KVH * 132], BF16, tag="vaug")
            nc.any.memset(vaug[:], 1.0)
            attnT = rp.tile([128, (QD // 128) * S], BF16, tag="attnT")

            # ================= projections + rope =================
            with tc.tile_pool(name="pj", bufs=4, space="PSUM") as pj:

                def rope_chunk(ps_qk, dst, dst_col):
                    """ps_qk: PSUM [128, S] fp32 pre-rope; write bf16 rope output
                    to dst[:, dst_col:dst_col+S]."""
                    raw = wp.tile([128, S], BF16, tag="raw")
                    nc.scalar.activation(
                        raw[:], ps_qk[:], mybir.ActivationFunctionType.Copy
                    )
                    ps_rot = pj.tile([128, S], F32, tag="pj")
                    for ns in range(2):
                        nc.tensor.matmul(
                            ps_rot[:, ns * 512:(ns + 1) * 512],
                            permT[:],
                            raw[:, ns * 512:(ns + 1) * 512],
                            start=True, stop=True,
                        )
                    t1 = wp.tile([128, S], F32, tag="t1")
                    nc.vector.tensor_tensor(
                        t1[:], raw[:], cos2[:], mybir.AluOpType.mult
                    )
                    t2 = wp.tile([128, S], F32, tag="t2")
                    nc.vector.tensor_tensor(
                        t2[:], ps_rot[:], sin2[:], mybir.AluOpType.mult
                    )
                    nc.vector.tensor_tensor(
                        dst[:, dst_col:dst_col + S], t1[:], t2[:],
                        mybir.AluOpType.add,
                    )

                # q projection: 4 chunks of 128 q-dims
                for mc in range(QD // 128):
                    ps = pj.tile([128, S], F32, tag="pj")
                    for ns in range(2):
                        for k in range(HC):
                            nc.tensor.matmul(
                                ps[:, ns * 512:(ns + 1) * 512],
                                wq[k][:, mc * 128:(mc + 1) * 128],
                                hsT[k][:, ns * 512: ns * 512 + 512],
                                start=(k == 0), stop=(k == HC - 1),
                            )
                    rope_chunk(ps, qrot, mc * S)

                # k projection (each kv head replicated across both halves)
                for kv in range(KVH):
                    ps = pj.tile([128, S], F32, tag="pj")
                    for ns in range(2):
                        for k in range(HC):
                            base = k * KVH * 2 * HD + kv * 2 * HD * 1
                            nc.tensor.matmul(
                                ps[:, ns * 512:(ns + 1) * 512],
                                wk[:, base:base + 128],
                                hsT[k][:, ns * 512: ns * 512 + 512],
                                start=(k == 0), stop=(k == HC - 1),
                            )
                    rope_chunk(ps, krep, kv * S)

                # v projection as V^T (stationary wv -> cheap LDWEIGHTS), then
                # PE-transpose each tok chunk into natural V layout
                ps_vt = pj.tile([128, S], F32, tag="pj")
                for ns in range(2):
                    for k in range(HC):
                        nc.tensor.matmul(
                            ps_vt[:, ns * 512:(ns + 1) * 512],
                            wv[:, k * KVH * HD:(k + 1) * KVH * HD],
                            hsT[k][:, ns * 512: ns * 512 + 512],
                            start=(k == 0), stop=(k == HC - 1),
                        )
                vt_sb = wp.tile([128, S], BF16, tag="vts")
                nc.scalar.activation(
                    vt_sb[:], ps_vt[:], mybir.ActivationFunctionType.Copy
                )
                for t in range(TC):
                    ps_tr = pj.tile([128, 128], BF16, tag="pj")
                    nc.tensor.transpose(
                        ps_tr[:], vt_sb[:, t * 128:(t + 1) * 128], ident[:]
                    )
                    for kv in range(KVH):
                        base = t * KVH * 132 + kv * 132
                        nc.vector.tensor_copy(
                            vaug[:, base:base + 64],
                            ps_tr[:, kv * 64:(kv + 1) * 64],
                        )
                        nc.vector.tensor_copy(
                            vaug[:, base + 65:base + 129],
                            ps_tr[:, kv * 64:(kv + 1) * 64],
                        )

            # ================= attention =================
            with (
                tc.tile_pool(name="st", bufs=2, space="PSUM") as stp,
                tc.tile_pool(name="av", bufs=2, space="PSUM") as avp,
            ):
                for h in range(QH):
                    kv = h // (QH // KVH)        # local kv head
                    mc = h // 2                  # q chunk
                    par = h % 2
                    r = par * 64                 # partition row base
                    # odd heads use psum rows 64:128 for attn, so row 0 is free
                    # to accumulate their softmax denominators
                    ps_at = avp.tile([128, S], F32, tag="av")
                    for kc in range(KC):
                        ps_st = stp.tile([128, S], F32, tag="st")
                        for ns in range(2):
                            nc.tensor.matmul(
                                ps_st[:, ns * 512:(ns + 1) * 512],
                                krep[r:r + 64, kv * S + kc * 128: kv * S + (kc + 1) * 128],
                                qrot[r:r + 64, mc * S + ns * 512: mc * S + ns * 512 + 512],
                                start=True, stop=True,
                            )
                        if use_mask:
                            nc.vector.tensor_tensor(
                                ps_st[:], ps_st[:],
                                maskT[:, kc * S:(kc + 1) * S],
                                mybir.AluOpType.add,
                            )
                        ex = ep.tile([128, S], BF16, tag="ex")
                        nc.scalar.activation(
                            ex[:], ps_st[:], mybir.ActivationFunctionType.Exp
                        )
                        # even heads: augmented PV lhsT [V|1] puts softmax
                        # denominators on psum row 64; odd heads (rows 64:128)
                        # can't fit the extra row, so they use a ones-matmul.
                        base = kc * KVH * 132 + kv * 132
                        m = 64 + (0 if par else 1)
                        for ns in range(2):
                            nc.tensor.matmul(
                                ps_at[r:r + m, ns * 512:(ns + 1) * 512],
                                vaug[:, base:base + m],
                                ex[:, ns * 512:(ns + 1) * 512],
                                start=(kc == 0), stop=(kc == KC - 1),
                            )
                            if par:
                                nc.tensor.matmul(
                                    ps_at[0:1, ns * 512:(ns + 1) * 512],
                                    ones_col[:],
                                    ex[:, ns * 512:(ns + 1) * 512],
                                    start=(kc == 0), stop=(kc == KC - 1),
                                )
                    if par:
                        recip0 = wp.tile([1, S], F32, tag="recip0")
                        nc.vector.reciprocal(recip0[:], ps_at[0:1, :])
                    else:
                        reciprow = wp.tile([128, S], F32, tag="reciprow")
                        nc.vector.reciprocal(
                            reciprow[64:65, :], ps_at[64:65, :]
                        )
                        recip0 = wp.tile([1, S], F32, tag="recip0")
                        nc.sync.dma_start(
                            out=recip0[0:1, :], in_=reciprow[64:65, :]
                        )
                    bcast = wp.tile([128, S], F32, tag="bcast")
                    nc.gpsimd.partition_broadcast(bcast[:], recip0[:])
                    nc.vector.tensor_tensor(
                        attnT[r:r + 64, mc * S:(mc + 1) * S],
                        ps_at[r:r + 64, :], bcast[r:r + 64, :],
                        mybir.AluOpType.mult,
                    )

            # ================= output projection (transposed out) =================
            with tc.tile_pool(name="wop", bufs=3, space="PSUM") as wop:
                for mc2 in range(HID // 128):
                    pso = wop.tile([128, S], F32, tag="wop")
                    for ns in range(2):
                        for kc2 in range(QD // 128):
                            nc.tensor.matmul(
                                pso[:, ns * 512:(ns + 1) * 512],
                                wo[:, kc2 * HID + mc2 * 128: kc2 * HID + (mc2 + 1) * 128],
                                attnT[:, kc2 * S + ns * 512: kc2 * S + ns * 512 + 512],
                                start=(kc2 == 0), stop=(kc2 == QD // 128 - 1),
                            )
                    osb = op_.tile([128, S], F32, tag="osb")
                    nc.vector.tensor_copy(osb[:], pso[:])
                    nc.sync.dma_start(
                        out=out_d[mc2 * 128:(mc2 + 1) * 128, :], in_=osb[:]
                    )
    nc.finalize()
    return nc


def _rope_tables():
    inv = 1.0 / (ROPE_BASE ** (np.arange(0, HD, 2, dtype=np.float32) / HD))
    t = np.arange(S, dtype=np.float32)
    freqs = np.outer(t, inv)
    emb = np.concatenate([freqs, freqs], axis=-1)  # [S, HD]
    return np.cos(emb).astype(np.float32), np.sin(emb).astype(np.float32)


def _perm_T():
    P = np.zeros((128, 128), dtype=np.float32)
    for blk in range(2):
        o = blk * 64
        for i in range(32):
            P[o + i, o + i + 32] = -1.0
            P[o + i + 32, o + i] = 1.0
    return P.T.astype(ml_dtypes.bfloat16)


def kernel(hidden_states, position_ids, attention_mask, Wq, Wk, Wv, Wo,
           _trace=False):
    global LAST_RESULT
    bf = ml_dtypes.bfloat16
    hidden_states = np.asarray(hidden_states, dtype=np.float32)
    Wq = np.asarray(Wq, dtype=np.float32)
    Wk = np.asarray(Wk, dtype=np.float32)
    Wv = np.asarray(Wv, dtype=np.float32)
    Wo = np.asarray(Wo, dtype=np.float32)
    mask = np.asarray(attention_mask, dtype=np.float32)
    pos = np.asarray(position_ids).astype(np.int64)

    use_mask = bool(np.any(mask))
    key = use_mask
    if key not in _CACHE:
        _CACHE[key] = _build(use_mask)
    nc = _CACHE[key]

    cos_t, sin_t = _rope_tables()
    permT = _perm_T()
    scale = 1.0 / np.sqrt(HD)

    in_maps = []
    for c in range(8):
        b, g = c // G, c % G
        m = {
            "hsT": np.ascontiguousarray(hidden_states[b].T).astype(bf),
            "wq": (Wq[:, g * QD:(g + 1) * QD] * scale).astype(bf),
            "wv": Wv[:, g * KVH * HD:(g + 1) * KVH * HD].astype(bf),
            "wo": Wo[g * QD:(g + 1) * QD, :].astype(bf),
            "permT": permT,
        }
        # kv heads replicated into both 64-row halves of each 128-col chunk
        wk_parts = []
        for kv in range(KVH):
            col = (g * KVH + kv) * HD
            wk_parts += [Wk[:, col:col + HD], Wk[:, col:col + HD]]
        m["wk"] = np.concatenate(wk_parts, axis=1).astype(bf)
        cos_b = cos_t[pos[b]]                      # [S, HD]
        m["cos2"] = np.ascontiguousarray(
            np.tile(cos_b.T, (2, 1))).astype(np.float32)
        m["sin2"] = np.ascontiguousarray(
            np.tile(sin_t[pos[b]].T, (2, 1))).astype(np.float32)
        if use_mask:
            mt = np.maximum(mask[b, 0], NEG_BIG).T    # [S(k), S(q)]
            m["maskT"] = np.ascontiguousarray(mt).astype(bf)
        in_maps.append(m)

    res = run_bass_kernel_spmd(nc, in_maps, core_ids=list(range(8)),
                               trace=_trace)
    LAST_RESULT = res
    out = np.zeros((B, S, HID), dtype=np.float32)
    for c in range(8):
        out[c // G] += res.results[c]["out"].T
    return out



# revision 7
# speedup vs baseline: 1.4595x; 1.4595x over previous
"""GQA attention (B=2,S=1024,HID=2048,NH=32,NKV=8,HD=64) on 8 TRN2 cores.

Sharding: core c -> batch b=c//4, head-group g=c%4 (8 q heads / 2 kv heads).
Within a core, q heads are re-paired as (m, m+4) for m in 0..3 so that the
head using local kv0 sits at partitions 0:64 and the head using kv1 at
64:128.  This makes the scores matmuls a row-tiled concurrent pair (64x128
array tiles) with an UN-replicated K, and the PV matmuls a col-tiled
concurrent pair (128x64 tiles) writing both heads into one PSUM tile.

Softmax denominators: exp chunks are tree-summed across k-chunks on the DVE
(bf16), reduced across partitions by one ones-matmul per 512 q columns, and
inverted with reciprocal_approx_fast.  gpsimd broadcasts the reciprocal row;
DVE applies it while writing attnT.

Pipelining: the exp stream (ScalarE, the throughput floor at ~1.15us per
[128,1024] tile) starts as soon as K+Q0 projections finish trailing the
input DMA.  V and Q1-3 projections are emitted as PE "filler" between the
scores/PV matmuls of earlier pairs, so the tensor engine works under the
exp stream instead of before it.  Wo runs at the end (128 N=512 matmuls).

PSUM budget (8 banks): tag "pv" [128,1024]f32 x2 bufs (4 banks) rotates
K/Q0..Q3/V accumulators, per-pair PV accumulators and Wo accumulators;
tag "sc" [128,1024]f32 x2 bufs (4 banks) rotates score tiles, rope rotate
scratch and denominator tiles.
"""

import numpy as np
import ml_dtypes

import concourse.bass as bass
import concourse.bacc as bacc
import concourse.mybir as mybir
from concourse.tile import TileContext
from concourse.bass_utils import run_bass_kernel_spmd

B, S, HID = 2, 1024, 2048
NH, NKV, HD = 32, 8, 64
G = 4                      # head groups (tensor-parallel degree per batch)
QH = NH // G               # 8 q heads per core
KVH = NKV // G             # 2 kv heads per core
QD = QH * HD               # 512
HC = HID // 128            # 16 hidden chunks
KC = S // 128              # 8 k-token chunks
ROPE_BASE = 10000.0
BF16 = mybir.dt.bfloat16
F32 = mybir.dt.float32
NEG_BIG = float(np.finfo(np.float32).min)
MULT = mybir.AluOpType.mult
ADD = mybir.AluOpType.add
EXP = mybir.ActivationFunctionType.Exp

LAST_RESULT = None
_CACHE = {}


def _build(use_mask: bool) -> bass.Bass:
    nc = bacc.Bacc(None, target_bir_lowering=False)
    hsT_d = nc.dram_tensor("hsT", [HID, S], BF16, kind="ExternalInput")
    wq_d = nc.dram_tensor("wq", [HID, QD], BF16, kind="ExternalInput")
    wk_d = nc.dram_tensor("wk", [HID, KVH * HD], BF16, kind="ExternalInput")
    wv_d = nc.dram_tensor("wv", [HID, KVH * HD], BF16, kind="ExternalInput")
    wo_d = nc.dram_tensor("wo", [QD, HID], BF16, kind="ExternalInput")
    cos_d = nc.dram_tensor("cos2", [128, S], F32, kind="ExternalInput")
    sin_d = nc.dram_tensor("sin2", [128, S], F32, kind="ExternalInput")
    perm_d = nc.dram_tensor("permT", [128, 128], BF16, kind="ExternalInput")
    if use_mask:
        mask_d = nc.dram_tensor("maskT", [S, S], BF16, kind="ExternalInput")
    out_d = nc.dram_tensor("out", [HID, S], F32, kind="ExternalOutput")

    with TileContext(nc) as tc:
        with (
            tc.tile_pool(name="const", bufs=1) as cp,
            tc.tile_pool(name="work", bufs=2) as wp,
            tc.tile_pool(name="ps", bufs=2, space="PSUM") as pp,
        ):
            # warm the exp table + custom-DVE ucode during the DMA window
            dmy = cp.tile([1, 8], F32, tag="dmy")
            nc.any.memset(dmy[:], 1.0)
            dmye = cp.tile([1, 8], BF16, tag="dmye")
            nc.scalar.activation(dmye[:], dmy[:], EXP)
            dmyr = cp.tile([1, 8], F32, tag="dmyr")
            nc.vector.reciprocal_approx_fast(dmyr[:], dmy[:])

            ones_col = cp.tile([128, 1], BF16, tag="ones")
            nc.any.memset(ones_col[:], 1.0)

            # ---- input DMAs, ordered for earliest first-scores ----
            permT = cp.tile([128, 128], BF16, tag="permT")
            nc.sync.dma_start(out=permT[:], in_=perm_d[:, :])
            wkc = cp.tile([128, HC * 128], BF16, tag="wkc")
            nc.sync.dma_start(
                out=wkc[:].rearrange("p (k m) -> p k m", k=HC),
                in_=wk_d[:, :].rearrange("(k p) m -> p k m", p=128),
            )
            wqc = []
            for m in range(4):
                wqc.append(cp.tile([128, HC * 128], BF16, tag=f"wq{m}",
                                   name=f"wq{m}"))
            nc.sync.dma_start(
                out=wqc[0][:].rearrange("p (k m) -> p k m", k=HC),
                in_=wq_d[:, 0:128].rearrange("(k p) m -> p k m", p=128),
            )
            cos2 = cp.tile([128, S], F32, tag="cos2")
            nc.sync.dma_start(out=cos2[:], in_=cos_d[:, :])
            sin2 = cp.tile([128, S], F32, tag="sin2")
            nc.sync.dma_start(out=sin2[:], in_=sin_d[:, :])
            hsT = []
            for k in range(HC):
                hsT.append(cp.tile([128, S], BF16, tag=f"hsT{k}",
                                   name=f"hsT{k}"))
            for k in range(8):
                nc.sync.dma_start(out=hsT[k][:], in_=hsT_d[k * 128:(k + 1) * 128, :])
            wvc = cp.tile([128, HC * 128], BF16, tag="wvc")
            nc.sync.dma_start(
                out=wvc[:].rearrange("p (k m) -> p k m", k=HC),
                in_=wv_d[:, :].rearrange("(k p) m -> p k m", p=128),
            )
            for k in range(8, HC):
                nc.sync.dma_start(out=hsT[k][:], in_=hsT_d[k * 128:(k + 1) * 128, :])
            for m in range(1, 4):
                nc.sync.dma_start(
                    out=wqc[m][:].rearrange("p (k m) -> p k m", k=HC),
                    in_=wq_d[:, m * 128:(m + 1) * 128].rearrange(
                        "(k p) m -> p k m", p=128),
                )
            woc = cp.tile([128, 4 * HID], BF16, tag="woc")
            nc.sync.dma_start(
                out=woc[:].rearrange("p (k m) -> p k m", k=4),
                in_=wo_d[:, :].rearrange("(k p) m -> p k m", p=128),
            )
            if use_mask:
                maskT = cp.tile([128, KC * S], BF16, tag="maskT")
                nc.sync.dma_start(
                    out=maskT[:].rearrange("p (k q) -> p k q", k=KC),
                    in_=mask_d[:, :].rearrange("(k p) q -> p k q", p=128),
                )

            # ---- persistent intermediates ----
            krot = cp.tile([128, S], BF16, tag="krot")
            qrot = cp.tile([128, 4 * S], BF16, tag="qrot")
            vtmp = cp.tile([128, S], BF16, tag="vtmp")
            vnat = cp.tile([128, S], BF16, tag="vnat")
            attnT = cp.tile([128, 4 * S], BF16, tag="attnT")

            def rope(ps, dst):
                """ps: PSUM [128, S] f32 pre-rope; dst: SBUF bf16 AP [128, S]."""
                for ns in range(2):
                    sl = slice(ns * 512, (ns + 1) * 512)
                    raw = wp.tile([128, 512], BF16, tag="raw")
                    nc.vector.tensor_copy(raw[:], ps[:, sl])
                    rot = pp.tile([128, 512], F32, tag="sc")
                    nc.tensor.matmul(rot[:], permT[:], raw[:],
                                     start=True, stop=True)
                    t1 = wp.tile([128, 512], F32, tag="t1")
                    nc.vector.tensor_tensor(t1[:], raw[:], cos2[:, sl], MULT)
                    t2 = wp.tile([128, 512], F32, tag="t2")
                    nc.vector.tensor_tensor(t2[:], rot[:], sin2[:, sl], MULT)
                    nc.vector.tensor_tensor(dst[:, sl], t1[:], t2[:], ADD)

            # ---- K + Q0 projections, interleaved, trailing the hsT DMA ----
            kps = pp.tile([128, S], F32, tag="pv")
            q0ps = pp.tile([128, S], F32, tag="pv")
            for k in range(HC):
                for ns in range(2):
                    nc.tensor.matmul(
                        kps[:, ns * 512:(ns + 1) * 512],
                        wkc[:, k * 128:(k + 1) * 128],
                        hsT[k][:, ns * 512:(ns + 1) * 512],
                        start=(k == 0), stop=(k == HC - 1),
                    )
                for ns in range(2):
                    nc.tensor.matmul(
                        q0ps[:, ns * 512:(ns + 1) * 512],
                        wqc[0][:, k * 128:(k + 1) * 128],
                        hsT[k][:, ns * 512:(ns + 1) * 512],
                        start=(k == 0), stop=(k == HC - 1),
                    )
            rope(kps, krot[:])
            rope(q0ps, qrot[:, 0:S])

            # ---- PE filler units (run under the exp stream) ----
            state = {}

            def v_unit(k):
                def emit():
                    if "vps" not in state:
                        state["vps"] = pp.tile([128, S], F32, tag="pv",
                                               name="vps")
                    vps = state["vps"]
                    for ns in range(2):
                        nc.tensor.matmul(
                            vps[:, ns * 512:(ns + 1) * 512],
                            wvc[:, k * 128:(k + 1) * 128],
                            hsT[k][:, ns * 512:(ns + 1) * 512],
                            start=(k == 0), stop=(k == HC - 1),
                        )
                return emit

            def vnat_unit():
                def emit():
                    nc.vector.tensor_copy(vtmp[:], state["vps"][:])
                    for t in range(KC):
                        nc.sync.dma_start_transpose(
                            vnat[:, t * 128:(t + 1) * 128],
                            vtmp[:, t * 128:(t + 1) * 128],
                        )
                return emit

            def q_unit(m, k):
                def emit():
                    key = f"qps{m}"
                    if key not in state:
                        state[key] = pp.tile([128, S], F32, tag="pv", name=key)
                    qps = state[key]
                    for ns in range(2):
                        nc.tensor.matmul(
                            qps[:, ns * 512:(ns + 1) * 512],
                            wqc[m][:, k * 128:(k + 1) * 128],
                            hsT[k][:, ns * 512:(ns + 1) * 512],
                            start=(k == 0), stop=(k == HC - 1),
                        )
                return emit

            def qrope_unit(m):
                def emit():
                    rope(state[f"qps{m}"], qrot[:, m * S:(m + 1) * S])
                return emit

            filler = {0: [], 1: [], 2: [], 3: []}
            for k in range(HC):
                filler[0].append(v_unit(k))
            filler[0].append(vnat_unit())
            for m in range(1, 4):
                for k in range(HC):
                    filler[m - 1].append(q_unit(m, k))
                filler[m - 1].append(qrope_unit(m))

            def drain(m, n):
                q = filler[m]
                for _ in range(min(n, len(q))):
                    q.pop(0)()

            # ---- attention ----
            exs = {}

            def pv(m, kc, psO):
                exA, exB = exs[(m, kc)]
                for ns in range(2):
                    nc.tensor.matmul(
                        psO[0:64, ns * 512:(ns + 1) * 512],
                        vnat[:, kc * 128:kc * 128 + 64],
                        exA[:, ns * 512:(ns + 1) * 512],
                        start=(kc == 0), stop=(kc == KC - 1),
                    )
                for ns in range(2):
                    nc.tensor.matmul(
                        psO[64:128, ns * 512:(ns + 1) * 512],
                        vnat[:, kc * 128 + 64:(kc + 1) * 128],
                        exB[:, ns * 512:(ns + 1) * 512],
                        start=(kc == 0), stop=(kc == KC - 1),
                    )

            for m in range(4):
                psO = None
                u1 = {}
                u3 = {}
                for kc in range(KC):
                    scA = pp.tile([128, S], F32, tag="sc")
                    scB = pp.tile([128, S], F32, tag="sc")
                    for ns in range(2):
                        nc.tensor.matmul(
                            scA[:, ns * 512:(ns + 1) * 512],
                            krot[0:64, kc * 128:(kc + 1) * 128],
                            qrot[0:64, m * S + ns * 512:m * S + ns * 512 + 512],
                            start=True, stop=True,
                        )
                    for ns in range(2):
                        nc.tensor.matmul(
                            scB[:, ns * 512:(ns + 1) * 512],
                            krot[64:128, kc * 128:(kc + 1) * 128],
                            qrot[64:128, m * S + ns * 512:m * S + ns * 512 + 512],
                            start=True, stop=True,
                        )
                    if use_mask:
                        nc.vector.tensor_tensor(
                            scA[:], scA[:], maskT[:, kc * S:(kc + 1) * S], ADD)
                        nc.vector.tensor_tensor(
                            scB[:], scB[:], maskT[:, kc * S:(kc + 1) * S], ADD)
                    exA = wp.tile([128, S], BF16, tag="ex", bufs=16)
                    nc.scalar.activation(exA[:], scA[:], EXP)
                    exB = wp.tile([128, S], BF16, tag="ex", bufs=16)
                    nc.scalar.activation(exB[:], scB[:], EXP)
                    exs[(m, kc)] = (exA, exB)

                    # incremental bf16 tree-sum of exp chunks (for denominators)
                    if kc in (1, 5):
                        tgt = u1 if kc == 1 else u3
                        for h in range(2):
                            t = wp.tile([128, S], BF16, tag="tt", bufs=8)
                            nc.vector.tensor_tensor(
                                t[:], exs[(m, kc - 1)][h][:], exs[(m, kc)][h][:],
                                ADD)
                            tgt[h] = t
                    if kc in (3, 7):
                        tgt = u1 if kc == 3 else u3
                        for h in range(2):
                            t = wp.tile([128, S], BF16, tag="tt", bufs=8)
                            nc.vector.tensor_tensor(
                                t[:], exs[(m, kc - 1)][h][:], exs[(m, kc)][h][:],
                                ADD)
                            nc.vector.tensor_tensor(
                                tgt[h][:], tgt[h][:], t[:], ADD)
                    if kc == 7:
                        for h in range(2):
                            nc.vector.tensor_tensor(
                                u1[h][:], u1[h][:], u3[h][:], ADD)

                    # PV lags the exp stream; pair 0 also waits for V/vnat
                    if m == 0:
                        if kc == 6:
                            psO = pp.tile([128, S], F32, tag="pv")
                            for j in range(3):
                                pv(m, j, psO)
                        elif kc == 7:
                            for j in range(3, 6):
                                pv(m, j, psO)
                    elif kc >= 2:
                        if psO is None:
                            psO = pp.tile([128, S], F32, tag="pv")
                        pv(m, kc - 2, psO)

                    drain(m, 3)

                drain(m, len(filler[m]))
                for j in range(6, KC):
                    pv(m, j, psO)

                # denominators -> reciprocals -> broadcast -> normalize
                rcs = []
                for h in range(2):
                    for half in range(2):
                        dn = pp.tile([1, 512], F32, tag="sc")
                        nc.tensor.matmul(
                            dn[:], ones_col[:],
                            u1[h][:, half * 512:(half + 1) * 512],
                            start=True, stop=True,
                        )
                        rc = wp.tile([1, 512], F32, tag="rc", bufs=4)
                        nc.vector.reciprocal_approx_fast(rc[:], dn[:])
                        rcs.append(rc)
                for h in range(2):
                    bc = wp.tile([128, S], F32, tag="bc")
                    nc.gpsimd.partition_broadcast(bc[:, 0:512], rcs[2 * h][:])
                    nc.gpsimd.partition_broadcast(
                        bc[:, 512:1024], rcs[2 * h + 1][:])
                    r = h * 64
                    nc.vector.tensor_tensor(
                        attnT[r:r + 64, m * S:(m + 1) * S],
                        psO[r:r + 64, :], bc[r:r + 64, :], MULT)

            # ---- output projection ----
            for mc2 in range(HC):
                psW = pp.tile([128, S], F32, tag="pv")
                for ns in range(2):
                    for mm in range(4):
                        nc.tensor.matmul(
                            psW[:, ns * 512:(ns + 1) * 512],
                            woc[:, mm * HID + mc2 * 128:mm * HID + (mc2 + 1) * 128],
                            attnT[:, mm * S + ns * 512:mm * S + ns * 512 + 512],
                            start=(mm == 0), stop=(mm == 3),
                        )
                outst = wp.tile([128, S], F32, tag="os")
                nc.vector.tensor_copy(outst[:], psW[:])
                nc.sync.dma_start(
                    out=out_d[mc2 * 128:(mc2 + 1) * 128, :], in_=outst[:])
    nc.finalize()
    return nc


def _rope_tables():
    inv = 1.0 / (ROPE_BASE ** (np.arange(0, HD, 2, dtype=np.float32) / HD))
    t = np.arange(S, dtype=np.float32)
    freqs = np.outer(t, inv)
    emb = np.concatenate([freqs, freqs], axis=-1)  # [S, HD]
    return np.cos(emb).astype(np.float32), np.sin(emb).astype(np.float32)


def _perm_T():
    P = np.zeros((128, 128), dtype=np.float32)
    for blk in range(2):
        o = blk * 64
        for i in range(32):
            P[o + i, o + i + 32] = -1.0
            P[o + i + 32, o + i] = 1.0
    return P.T.astype(ml_dtypes.bfloat16)


def kernel(hidden_states, position_ids, attention_mask, Wq, Wk, Wv, Wo,
           _trace=False):
    global LAST_RESULT
    bf = ml_dtypes.bfloat16
    hidden_states = np.asarray(hidden_states, dtype=np.float32)
    Wq = np.asarray(Wq, dtype=np.float32)
    Wk = np.asarray(Wk, dtype=np.float32)
    Wv = np.asarray(Wv, dtype=np.float32)
    Wo = np.asarray(Wo, dtype=np.float32)
    mask = np.asarray(attention_mask, dtype=np.float32)
    pos = np.asarray(position_ids).astype(np.int64)

    use_mask = bool(np.any(mask))
    if use_mask not in _CACHE:
        _CACHE[use_mask] = _build(use_mask)
    nc = _CACHE[use_mask]

    cos_t, sin_t = _rope_tables()
    permT = _perm_T()
    scale = 1.0 / np.sqrt(HD)

    in_maps = []
    for c in range(8):
        b, g = c // G, c % G
        # paired head order: chunk m holds (head 8g+m, head 8g+m+4)
        order = []
        for m in range(4):
            order += [8 * g + m, 8 * g + m + 4]
        wq_g = np.concatenate(
            [Wq[:, h * HD:(h + 1) * HD] for h in order], axis=1) * scale
        wo_g = np.concatenate(
            [Wo[h * HD:(h + 1) * HD, :] for h in order], axis=0)
        kv0 = 2 * g * HD
        m = {
            "hsT": np.ascontiguousarray(hidden_states[b].T).astype(bf),
            "wq": wq_g.astype(bf),
            "wk": np.ascontiguousarray(Wk[:, kv0:kv0 + 2 * HD]).astype(bf),
            "wv": np.ascontiguousarray(Wv[:, kv0:kv0 + 2 * HD]).astype(bf),
            "wo": np.ascontiguousarray(wo_g).astype(bf),
            "permT": permT,
            "cos2": np.ascontiguousarray(
                np.tile(cos_t[pos[b]].T, (2, 1))).astype(np.float32),
            "sin2": np.ascontiguousarray(
                np.tile(sin_t[pos[b]].T, (2, 1))).astype(np.float32),
        }
        if use_mask:
            mt = np.maximum(mask[b, 0], NEG_BIG).T    # [S(k), S(q)]
            m["maskT"] = np.ascontiguousarray(mt).astype(bf)
        in_maps.append(m)

    res = run_bass_kernel_spmd(nc, in_maps, core_ids=list(range(8)),
                               trace=_trace)
    LAST_RESULT = res
    out = np.zeros((B, S, HID), dtype=np.float32)
    for c in range(8):
        out[c // G] += res.results[c]["out"].T
    return out


# revision 13
# speedup vs baseline: 1.4619x; 1.0016x over previous
"""GQA attention (B=2,S=1024,HID=2048,NH=32,NKV=8,HD=64) on 8 TRN2 cores.

Sharding: core c -> batch b=c//4, head-group g=c%4 (8 q heads / 2 kv heads).
Within a core, q heads are re-paired as (m, m+4) for m in 0..3 so that the
head using local kv0 sits at partitions 0:64 and the head using kv1 at
64:128.  This makes the scores matmuls a row-tiled concurrent pair (64x128
array tiles) with an UN-replicated K, and the PV matmuls a col-tiled
concurrent pair (128x64 tiles) writing both heads into one PSUM tile.

Softmax denominators: exp chunks are tree-summed across k-chunks on the DVE
(bf16), reduced across partitions by one ones-matmul per 512 q columns, and
inverted with reciprocal_approx_fast.  gpsimd broadcasts the reciprocal row;
DVE applies it while writing attnT.

Pipelining: the exp stream (ScalarE, the throughput floor at ~1.15us per
[128,1024] tile) starts as soon as K+Q0 projections finish trailing the
input DMA.  V and Q1-3 projections are emitted as PE "filler" between the
scores/PV matmuls of earlier pairs, so the tensor engine works under the
exp stream instead of before it.  Wo runs at the end (128 N=512 matmuls).

PSUM budget (8 banks): tag "pv" [128,1024]f32 x2 bufs (4 banks) rotates
K/Q0..Q3/V accumulators, per-pair PV accumulators and Wo accumulators;
tag "sc" [128,1024]f32 x2 bufs (4 banks) rotates score tiles, rope rotate
scratch and denominator tiles.
"""

import numpy as np
import ml_dtypes

import concourse.bass as bass
import concourse.bacc as bacc
import concourse.mybir as mybir
from concourse.tile import TileContext
from concourse.bass_utils import run_bass_kernel_spmd

B, S, HID = 2, 1024, 2048
NH, NKV, HD = 32, 8, 64
G = 4                      # head groups (tensor-parallel degree per batch)
QH = NH // G               # 8 q heads per core
KVH = NKV // G             # 2 kv heads per core
QD = QH * HD               # 512
HC = HID // 128            # 16 hidden chunks
KC = S // 128              # 8 k-token chunks
ROPE_BASE = 10000.0
BF16 = mybir.dt.bfloat16
F32 = mybir.dt.float32
NEG_BIG = float(np.finfo(np.float32).min)
MULT = mybir.AluOpType.mult
ADD = mybir.AluOpType.add
EXP = mybir.ActivationFunctionType.Exp

LAST_RESULT = None
_CACHE = {}


def _build(use_mask: bool) -> bass.Bass:
    nc = bacc.Bacc(None, target_bir_lowering=False)
    hsT_d = nc.dram_tensor("hsT", [HID, S], BF16, kind="ExternalInput")
    wq_d = nc.dram_tensor("wq", [HID, QD], BF16, kind="ExternalInput")
    wk_d = nc.dram_tensor("wk", [HID, KVH * HD], BF16, kind="ExternalInput")
    wv_d = nc.dram_tensor("wv", [HID, KVH * HD], BF16, kind="ExternalInput")
    wo_d = nc.dram_tensor("wo", [QD, HID], BF16, kind="ExternalInput")
    cos_d = nc.dram_tensor("cos2", [128, S], F32, kind="ExternalInput")
    sin_d = nc.dram_tensor("sin2", [128, S], F32, kind="ExternalInput")
    perm_d = nc.dram_tensor("permT", [128, 128], BF16, kind="ExternalInput")
    if use_mask:
        mask_d = nc.dram_tensor("maskT", [S, S], BF16, kind="ExternalInput")
    out_d = nc.dram_tensor("out", [HID, S], F32, kind="ExternalOutput")

    with TileContext(nc) as tc:
        with (
            tc.tile_pool(name="const", bufs=1) as cp,
            tc.tile_pool(name="work", bufs=2) as wp,
            tc.tile_pool(name="ps", bufs=2, space="PSUM") as pp,
        ):
            # warm the exp table + custom-DVE ucode during the DMA window
            dmy = cp.tile([1, 8], F32, tag="dmy")
            nc.any.memset(dmy[:], 1.0)
            dmye = cp.tile([1, 8], BF16, tag="dmye")
            nc.scalar.activation(dmye[:], dmy[:], EXP)
            dmyr = cp.tile([1, 8], F32, tag="dmyr")
            nc.vector.reciprocal_approx_fast(dmyr[:], dmy[:])

            ones_col = cp.tile([128, 1], BF16, tag="ones")
            nc.any.memset(ones_col[:], 1.0)

            # ---- input DMAs, ordered for earliest first-scores ----
            permT = cp.tile([128, 128], BF16, tag="permT")
            nc.sync.dma_start(out=permT[:], in_=perm_d[:, :])
            wkc = cp.tile([128, HC * 128], BF16, tag="wkc")
            nc.sync.dma_start(
                out=wkc[:].rearrange("p (k m) -> p k m", k=HC),
                in_=wk_d[:, :].rearrange("(k p) m -> p k m", p=128),
            )
            wqc = []
            for m in range(4):
                wqc.append(cp.tile([128, HC * 128], BF16, tag=f"wq{m}",
                                   name=f"wq{m}"))
            nc.sync.dma_start(
                out=wqc[0][:].rearrange("p (k m) -> p k m", k=HC),
                in_=wq_d[:, 0:128].rearrange("(k p) m -> p k m", p=128),
            )
            # second HWDGE queue (Scalar engine) for everything hsT doesn't gate
            cos2 = cp.tile([128, S], F32, tag="cos2")
            nc.scalar.dma_start(out=cos2[:], in_=cos_d[:, :])
            sin2 = cp.tile([128, S], F32, tag="sin2")
            nc.scalar.dma_start(out=sin2[:], in_=sin_d[:, :])
            wvc = cp.tile([128, HC * 128], BF16, tag="wvc")
            nc.scalar.dma_start(
                out=wvc[:].rearrange("p (k m) -> p k m", k=HC),
                in_=wv_d[:, :].rearrange("(k p) m -> p k m", p=128),
            )
            for m in range(1, 4):
                nc.scalar.dma_start(
                    out=wqc[m][:].rearrange("p (k m) -> p k m", k=HC),
                    in_=wq_d[:, m * 128:(m + 1) * 128].rearrange(
                        "(k p) m -> p k m", p=128),
                )
            if use_mask:
                maskT = cp.tile([128, KC * S], BF16, tag="maskT")
                nc.scalar.dma_start(
                    out=maskT[:].rearrange("p (k q) -> p k q", k=KC),
                    in_=mask_d[:, :].rearrange("(k p) q -> p k q", p=128),
                )
            hsT = []
            for k in range(HC):
                hsT.append(cp.tile([128, S], BF16, tag=f"hsT{k}",
                                   name=f"hsT{k}"))
            for k in range(HC):
                nc.sync.dma_start(out=hsT[k][:], in_=hsT_d[k * 128:(k + 1) * 128, :])
            woc = cp.tile([128, 4 * HID], BF16, tag="woc")
            nc.sync.dma_start(
                out=woc[:].rearrange("p (k m) -> p k m", k=4),
                in_=wo_d[:, :].rearrange("(k p) m -> p k m", p=128),
            )

            # ---- persistent intermediates ----
            krot = cp.tile([128, S], BF16, tag="krot")
            qrot = cp.tile([128, 4 * S], BF16, tag="qrot")
            vtmp = cp.tile([128, S], BF16, tag="vtmp")
            vnat = cp.tile([128, S], BF16, tag="vnat")
            attnT = cp.tile([128, 4 * S], BF16, tag="attnT")

            def rope(ps, dst):
                """ps: PSUM [128, S] f32 pre-rope; dst: SBUF bf16 AP [128, S]."""
                for ns in range(2):
                    sl = slice(ns * 512, (ns + 1) * 512)
                    raw = wp.tile([128, 512], BF16, tag="raw")
                    nc.vector.tensor_copy(raw[:], ps[:, sl])
                    rot = pp.tile([128, 512], F32, tag="sc")
                    nc.tensor.matmul(rot[:], permT[:], raw[:],
                                     start=True, stop=True)
                    t1 = wp.tile([128, 512], F32, tag="t1")
                    nc.vector.tensor_tensor(t1[:], raw[:], cos2[:, sl], MULT)
                    t2 = wp.tile([128, 512], F32, tag="t2")
                    nc.vector.tensor_tensor(t2[:], rot[:], sin2[:, sl], MULT)
                    nc.vector.tensor_tensor(dst[:, sl], t1[:], t2[:], ADD)

            # ---- K + Q0 projections, interleaved, trailing the hsT DMA ----
            kps = pp.tile([128, S], F32, tag="pv")
            q0ps = pp.tile([128, S], F32, tag="pv")
            for k in range(HC):
                for ns in range(2):
                    nc.tensor.matmul(
                        kps[:, ns * 512:(ns + 1) * 512],
                        wkc[:, k * 128:(k + 1) * 128],
                        hsT[k][:, ns * 512:(ns + 1) * 512],
                        start=(k == 0), stop=(k == HC - 1),
                    )
                for ns in range(2):
                    nc.tensor.matmul(
                        q0ps[:, ns * 512:(ns + 1) * 512],
                        wqc[0][:, k * 128:(k + 1) * 128],
                        hsT[k][:, ns * 512:(ns + 1) * 512],
                        start=(k == 0), stop=(k == HC - 1),
                    )
            rope(kps, krot[:])
            rope(q0ps, qrot[:, 0:S])

            # ---- PE filler units (run under the exp stream) ----
            state = {}

            def v_unit(k):
                def emit():
                    if "vps" not in state:
                        state["vps"] = pp.tile([128, S], F32, tag="pv",
                                               name="vps")
                    vps = state["vps"]
                    for ns in range(2):
                        nc.tensor.matmul(
                            vps[:, ns * 512:(ns + 1) * 512],
                            wvc[:, k * 128:(k + 1) * 128],
                            hsT[k][:, ns * 512:(ns + 1) * 512],
                            start=(k == 0), stop=(k == HC - 1),
                        )
                return emit

            def vnat_unit():
                def emit():
                    nc.vector.tensor_copy(vtmp[:], state["vps"][:])
                    for t in range(KC):
                        nc.sync.dma_start_transpose(
                            vnat[:, t * 128:(t + 1) * 128],
                            vtmp[:, t * 128:(t + 1) * 128],
                        )
                return emit

            def q_unit(m, k):
                def emit():
                    key = f"qps{m}"
                    if key not in state:
                        state[key] = pp.tile([128, S], F32, tag="pv", name=key)
                    qps = state[key]
                    for ns in range(2):
                        nc.tensor.matmul(
                            qps[:, ns * 512:(ns + 1) * 512],
                            wqc[m][:, k * 128:(k + 1) * 128],
                            hsT[k][:, ns * 512:(ns + 1) * 512],
                            start=(k == 0), stop=(k == HC - 1),
                        )
                return emit

            def qrope_unit(m):
                def emit():
                    rope(state[f"qps{m}"], qrot[:, m * S:(m + 1) * S])
                return emit

            filler = {0: [], 1: [], 2: [], 3: []}
            for k in range(HC):
                filler[0].append(v_unit(k))
            filler[0].append(vnat_unit())
            for m in range(1, 4):
                for k in range(HC):
                    filler[m - 1].append(q_unit(m, k))
                filler[m - 1].append(qrope_unit(m))

            def drain(m, n):
                q = filler[m]
                for _ in range(min(n, len(q))):
                    q.pop(0)()

            # ---- attention ----
            exs = {}

            def pv(m, kc, psO):
                exA, exB = exs[(m, kc)]
                for ns in range(2):
                    nc.tensor.matmul(
                        psO[0:64, ns * 512:(ns + 1) * 512],
                        vnat[:, kc * 128:kc * 128 + 64],
                        exA[:, ns * 512:(ns + 1) * 512],
                        start=(kc == 0), stop=(kc == KC - 1),
                    )
                for ns in range(2):
                    nc.tensor.matmul(
                        psO[64:128, ns * 512:(ns + 1) * 512],
                        vnat[:, kc * 128 + 64:(kc + 1) * 128],
                        exB[:, ns * 512:(ns + 1) * 512],
                        start=(kc == 0), stop=(kc == KC - 1),
                    )

            def finish_pair(m, psO, u1):
                """PV tail + denominators + normalize for pair m -> attnT."""
                for j in range(6, KC):
                    pv(m, j, psO)
                rcs = []
                for h in range(2):
                    for half in range(2):
                        dn = pp.tile([1, 512], F32, tag="sc")
                        nc.tensor.matmul(
                            dn[:], ones_col[:],
                            u1[h][:, half * 512:(half + 1) * 512],
                            start=True, stop=True,
                        )
                        rc = wp.tile([1, 512], F32, tag="rc", bufs=4)
                        nc.vector.reciprocal_approx_fast(rc[:], dn[:])
                        rcs.append(rc)
                for h in range(2):
                    bc = wp.tile([128, S], F32, tag="bc")
                    nc.gpsimd.partition_broadcast(bc[:, 0:512], rcs[2 * h][:])
                    nc.gpsimd.partition_broadcast(
                        bc[:, 512:1024], rcs[2 * h + 1][:])
                    r = h * 64
                    nc.vector.tensor_tensor(
                        attnT[r:r + 64, m * S:(m + 1) * S],
                        psO[r:r + 64, :], bc[r:r + 64, :], MULT)

            pending = None
            for m in range(4):
                psO = None
                u1 = {}
                u3 = {}
                for kc in range(KC):
                    scA = pp.tile([128, S], F32, tag="sc")
                    scB = pp.tile([128, S], F32, tag="sc")
                    for ns in range(2):
                        nc.tensor.matmul(
                            scA[:, ns * 512:(ns + 1) * 512],
                            krot[0:64, kc * 128:(kc + 1) * 128],
                            qrot[0:64, m * S + ns * 512:m * S + ns * 512 + 512],
                            start=True, stop=True,
                        )
                    for ns in range(2):
                        nc.tensor.matmul(
                            scB[:, ns * 512:(ns + 1) * 512],
                            krot[64:128, kc * 128:(kc + 1) * 128],
                            qrot[64:128, m * S + ns * 512:m * S + ns * 512 + 512],
                            start=True, stop=True,
                        )
                    if use_mask:
                        nc.vector.tensor_tensor(
                            scA[:], scA[:], maskT[:, kc * S:(kc + 1) * S], ADD)
                        nc.vector.tensor_tensor(
                            scB[:], scB[:], maskT[:, kc * S:(kc + 1) * S], ADD)
                    exA = wp.tile([128, S], BF16, tag="ex", bufs=16)
                    nc.scalar.activation(exA[:], scA[:], EXP)
                    exB = wp.tile([128, S], BF16, tag="ex", bufs=16)
                    nc.scalar.activation(exB[:], scB[:], EXP)
                    exs[(m, kc)] = (exA, exB)

                    # previous pair's tail runs after this pair's exp stream
                    # is already fed, keeping ScalarE busy across the seam
                    if kc == 0 and pending is not None:
                        finish_pair(*pending)
                        pending = None

                    # incremental bf16 tree-sum of exp chunks (for denominators)
                    if kc in (1, 5):
                        tgt = u1 if kc == 1 else u3
                        for h in range(2):
                            t = wp.tile([128, S], BF16, tag="tt", bufs=8)
                            nc.vector.tensor_tensor(
                                t[:], exs[(m, kc - 1)][h][:], exs[(m, kc)][h][:],
                                ADD)
                            tgt[h] = t
                    if kc in (3, 7):
                        tgt = u1 if kc == 3 else u3
                        for h in range(2):
                            t = wp.tile([128, S], BF16, tag="tt", bufs=8)
                            nc.vector.tensor_tensor(
                                t[:], exs[(m, kc - 1)][h][:], exs[(m, kc)][h][:],
                                ADD)
                            nc.vector.tensor_tensor(
                                tgt[h][:], tgt[h][:], t[:], ADD)
                    if kc == 7:
                        for h in range(2):
                            nc.vector.tensor_tensor(
                                u1[h][:], u1[h][:], u3[h][:], ADD)

                    # PV lags the exp stream; pair 0 also waits for V/vnat
                    if m == 0:
                        if kc == 6:
                            psO = pp.tile([128, S], F32, tag="pv")
                            for j in range(3):
                                pv(m, j, psO)
                        elif kc == 7:
                            for j in range(3, 6):
                                pv(m, j, psO)
                    elif kc >= 2:
                        if psO is None:
                            psO = pp.tile([128, S], F32, tag="pv")
                        pv(m, kc - 2, psO)

                    drain(m, 6 if m == 0 else 5)

                drain(m, len(filler[m]))
                pending = (m, psO, u1)

            finish_pair(*pending)

            # ---- output projection (ns-inner so consecutive matmuls
            # alternate PSUM banks and pipeline) ----
            for mc2 in range(HC):
                psW = pp.tile([128, S], F32, tag="pv")
                for mm in range(4):
                    for ns in range(2):
                        nc.tensor.matmul(
                            psW[:, ns * 512:(ns + 1) * 512],
                            woc[:, mm * HID + mc2 * 128:mm * HID + (mc2 + 1) * 128],
                            attnT[:, mm * S + ns * 512:mm * S + ns * 512 + 512],
                            start=(mm == 0), stop=(mm == 3),
                        )
                outst = wp.tile([128, S], F32, tag="os")
                nc.vector.tensor_copy(outst[:], psW[:])
                nc.sync.dma_start(
                    out=out_d[mc2 * 128:(mc2 + 1) * 128, :], in_=outst[:])
    nc.finalize()
    return nc


def _rope_tables():
    inv = 1.0 / (ROPE_BASE ** (np.arange(0, HD, 2, dtype=np.float32) / HD))
    t = np.arange(S, dtype=np.float32)
    freqs = np.outer(t, inv)
    emb = np.concatenate([freqs, freqs], axis=-1)  # [S, HD]
    return np.cos(emb).astype(np.float32), np.sin(emb).astype(np.float32)


def _perm_T():
    P = np.zeros((128, 128), dtype=np.float32)
    for blk in range(2):
        o = blk * 64
        for i in range(32):
            P[o + i, o + i + 32] = -1.0
            P[o + i + 32, o + i] = 1.0
    return P.T.astype(ml_dtypes.bfloat16)


def kernel(hidden_states, position_ids, attention_mask, Wq, Wk, Wv, Wo,
           _trace=False):
    global LAST_RESULT
    bf = ml_dtypes.bfloat16
    hidden_states = np.asarray(hidden_states, dtype=np.float32)
    Wq = np.asarray(Wq, dtype=np.float32)
    Wk = np.asarray(Wk, dtype=np.float32)
    Wv = np.asarray(Wv, dtype=np.float32)
    Wo = np.asarray(Wo, dtype=np.float32)
    mask = np.asarray(attention_mask, dtype=np.float32)
    pos = np.asarray(position_ids).astype(np.int64)

    use_mask = bool(np.any(mask))
    if use_mask not in _CACHE:
        _CACHE[use_mask] = _build(use_mask)
    nc = _CACHE[use_mask]

    cos_t, sin_t = _rope_tables()
    permT = _perm_T()
    scale = 1.0 / np.sqrt(HD)

    in_maps = []
    for c in range(8):
        b, g = c // G, c % G
        # paired head order: chunk m holds (head 8g+m, head 8g+m+4)
        order = []
        for m in range(4):
            order += [8 * g + m, 8 * g + m + 4]
        wq_g = np.concatenate(
            [Wq[:, h * HD:(h + 1) * HD] for h in order], axis=1) * scale
        wo_g = np.concatenate(
            [Wo[h * HD:(h + 1) * HD, :] for h in order], axis=0)
        kv0 = 2 * g * HD
        m = {
            "hsT": np.ascontiguousarray(hidden_states[b].T).astype(bf),
            "wq": wq_g.astype(bf),
            "wk": np.ascontiguousarray(Wk[:, kv0:kv0 + 2 * HD]).astype(bf),
            "wv": np.ascontiguousarray(Wv[:, kv0:kv0 + 2 * HD]).astype(bf),
            "wo": np.ascontiguousarray(wo_g).astype(bf),
            "permT": permT,
            "cos2": np.ascontiguousarray(
                np.tile(cos_t[pos[b]].T, (2, 1))).astype(np.float32),
            "sin2": np.ascontiguousarray(
                np.tile(sin_t[pos[b]].T, (2, 1))).astype(np.float32),
        }
        if use_mask:
            mt = np.maximum(mask[b, 0], NEG_BIG).T    # [S(k), S(q)]
            m["maskT"] = np.ascontiguousarray(mt).astype(bf)
        in_maps.append(m)

    res = run_bass_kernel_spmd(nc, in_maps, core_ids=list(range(8)),
                               trace=_trace)
    LAST_RESULT = res
    out = np.zeros((B, S, HID), dtype=np.float32)
    for c in range(8):
        out[c // G] += res.results[c]["out"].T
    return out
